# revision 27
# baseline (speedup 1.0000x reference)
"""Trainium2 Bass kernel for nn_AttentionOperation_32521492365427.

kernel(**inputs) -> np.ndarray, full shapes:
  query/key/value: [8, 8, 64, 1024] f32; gamma_sim/beta_sim: [8];
  gamma_val/beta_val: [512]; output: [8, 512, 1024] f32.

Sharded by HEAD across the 8 NeuronCores (one head per core): both
BatchNorms then have core-local statistics, so there are no collectives.

Per-core math:
 - softmax is shift-invariant => the sim-BN reduces to one per-head scale
   s = gamma_sim / sqrt(var(logits) + EPS); beta/mean drop out.
 - sumsq(logits_b) = sum(Gq_b * Gk_b) over 65x65 Gram matrices
   (ones column appended) => logits variance without a stats pass over
   the 8.4M logits.
 - softmax denominators come free as rows 64-95 of the PV matmul by
   appending 32 ones columns to V^T (the stationary operand); matmul cost
   is column-count-bound so the extra output rows are free.
 - denominator reciprocal is broadcast across partitions with two
   cross-quadrant stream_shuffles straight out of PSUM (pure DVE, f32).
 - val-BN affine + exact (erf) gelu fuse into a single ACT pass,
   split per batch and pipelined with the output DMAs.

All big inputs are pre-cast to bf16 on the host so DMAs are half-size
and issue from the hardware DGE queues (sync + scalar engines).
"""

import os
import sys

sys.path.insert(0, "/opt/trn_rl_repo")

from contextlib import ExitStack

import numpy as np
import ml_dtypes

import concourse.bacc as bacc
import concourse.bass as bass  # noqa: F401
import concourse.tile as tile
from concourse import mybir

F32 = mybir.dt.float32
BF16 = mybir.dt.bfloat16
FP8 = mybir.dt.float8e4
I32 = mybir.dt.int32
AF = mybir.ActivationFunctionType
OP = mybir.AluOpType
DR = mybir.MatmulPerfMode.DoubleRow

EPS = 1e-3
NB = 8
D = 64
C = 64
L = 1024
M = 1024
NCH = M // 128
NLM = float(NB * L * M)
MAGIC = 0x5F3759DF
MASK0 = [0] * 32


def _newton_rsqrt(nc, x, y, t, magic_i32, iters=2):
    """y = 1/sqrt(x) entirely on DVE (bit-trick seed + Newton iters)."""
    xi = x.bitcast(I32)
    yi = y.bitcast(I32)
    nc.vector.tensor_scalar(
        out=yi, in0=xi, scalar1=1, scalar2=None, op0=OP.arith_shift_right
    )
    nc.vector.tensor_tensor(out=yi, in0=magic_i32, in1=yi, op=OP.subtract)
    for _ in range(iters):
        nc.vector.tensor_mul(t, y, y)
        nc.vector.tensor_mul(t, t, x)
        nc.vector.tensor_scalar(
            out=t, in0=t, scalar1=-0.5, scalar2=1.5, op0=OP.mult, op1=OP.add
        )
        nc.vector.tensor_mul(y, y, t)


def build_nc(debug: bool = False):
    nc = bacc.Bacc("TRN2", target_bir_lowering=False, debug=debug)

    q2_d = nc.dram_tensor("q2", [128, NB // 2, L], BF16, kind="ExternalInput")
    k2_d = nc.dram_tensor("k2", [128, NB // 2, L], BF16, kind="ExternalInput")
    qt1_d = nc.dram_tensor("qt1", [128, NB, NCH * 65], FP8, kind="ExternalInput")
    kt1_d = nc.dram_tensor("kt1", [128, NB, NCH * 65], FP8, kind="ExternalInput")
    vt1_d = nc.dram_tensor("vt1", [128, NB, NCH * 66], BF16, kind="ExternalInput")
    gsim_d = nc.dram_tensor("g_sim", [1, 1], F32, kind="ExternalInput")
    gval_d = nc.dram_tensor("gamma_val", [C, 1], F32, kind="ExternalInput")
    bval_d = nc.dram_tensor("beta_val", [C, 1], F32, kind="ExternalInput")
    out_d = nc.dram_tensor("out", [NB, C, L], F32, kind="ExternalOutput")

    with tile.TileContext(nc) as tc, ExitStack() as ctx:
        const_p = ctx.enter_context(tc.tile_pool(name="const", bufs=1))
        pt_p = ctx.enter_context(tc.tile_pool(name="pt", bufs=4))
        bc_p = ctx.enter_context(tc.tile_pool(name="bc", bufs=2))
        big_p = ctx.enter_context(tc.tile_pool(name="big", bufs=1))
        small = ctx.enter_context(tc.tile_pool(name="small", bufs=1))

        # ---- constants / input staging ----
        ones_sb = const_p.tile([128, 128], F32, tag="ones")
        nc.vector.memset(ones_sb[:], 1.0)
        ones_bf = const_p.tile([64, 512], BF16, tag="onesbf")
        nc.gpsimd.memset(ones_bf[:], 1.0)
        magic_sb = const_p.tile([C, 1], I32, tag="magic")
        nc.vector.memset(magic_sb[:], MAGIC)
        dum = small.tile([1, 2], F32, tag="dum")
        nc.gpsimd.memset(dum[:], 0.5)

        qt_sb = const_p.tile([128, NB, NCH, 65], FP8, tag="qt")
        kt_sb = const_p.tile([128, NB, NCH, 65], FP8, tag="kt")
        vt_sb = const_p.tile([128, NB, NCH, 66], BF16, tag="vt")
        q2_sb = const_p.tile([128, NB // 2, L], BF16, tag="q2")
        k2_sb = const_p.tile([128, NB // 2, L], BF16, tag="k2")

        H = NB // 2
        # interleave issue engines: sync and scalar are the two HW DGE queues
        nc.sync.dma_start(out=qt_sb[:, 0:4], in_=qt1_d[:, 0:4])
        nc.scalar.dma_start(out=kt_sb[:, 0:4], in_=kt1_d[:, 0:4])
        nc.sync.dma_start(out=q2_sb[:, 0:1, :], in_=q2_d[:, 0:1])
        nc.scalar.dma_start(out=k2_sb[:, 0:1, :], in_=k2_d[:, 0:1])
        nc.sync.dma_start(out=qt_sb[:, 4:8], in_=qt1_d[:, 4:8])
        nc.scalar.dma_start(out=kt_sb[:, 4:8], in_=kt1_d[:, 4:8])
        nc.sync.dma_start(out=vt_sb[:], in_=vt1_d[:])
        nc.scalar.dma_start(out=q2_sb[:, 1:4, :], in_=q2_d[:, 1:4])
        nc.sync.dma_start(out=k2_sb[:, 1:4, :], in_=k2_d[:, 1:4])

        gsim_sb = const_p.tile([1, 1], F32, tag="gsim")
        nc.scalar.dma_start(out=gsim_sb[:], in_=gsim_d[:])
        gval_sb = const_p.tile([C, 1], F32, tag="gval")
        nc.scalar.dma_start(out=gval_sb[:], in_=gval_d[:])
        bval_sb = const_p.tile([C, 1], F32, tag="bval")
        nc.scalar.dma_start(out=bval_sb[:], in_=bval_d[:])

        # preload the EXP act table while DMAs are in flight
        dume = small.tile([1, 2], F32, tag="dume")
        nc.scalar.activation(dume[:, 0:1], dum[:, 0:1], AF.Exp)

        s_bcast = small.tile([128, 1], F32, tag="sbc")

        # ---- phase 0: Gram-matrix logits variance -> s ----
        with tc.tile_pool(name="gram", bufs=2, space="PSUM") as gram_p, tc.tile_pool(
            name="sf", bufs=1, space="PSUM"
        ) as sf_p:
            acc = small.tile([65, NB], F32, tag="acc")
            for b in range(NB):
                gq_ps = gram_p.tile([65, 65], F32, tag="gq")
                gk_ps = gram_p.tile([65, 65], F32, tag="gk")
                for c in range(NCH):
                    nc.tensor.matmul(
                        gq_ps[:],
                        qt_sb[:, b, c, :],
                        qt_sb[:, b, c, :],
                        start=(c == 0),
                        stop=(c == NCH - 1),
                    )
                for c in range(NCH):
                    nc.tensor.matmul(
                        gk_ps[:],
                        kt_sb[:, b, c, :],
                        kt_sb[:, b, c, :],
                        start=(c == 0),
                        stop=(c == NCH - 1),
                    )
                gq_sb = small.tile([65, 65], F32, tag="gq_sb")
                nc.scalar.copy(gq_sb[:], gq_ps[:])
                gk_sb = small.tile([65, 65], F32, tag="gk_sb")
                nc.vector.tensor_copy(gk_sb[:], gk_ps[:])
                prod = small.tile([65, 65], F32, tag="prod")
                nc.vector.tensor_mul(prod[:], gq_sb[:], gk_sb[:])
                nc.vector.reduce_sum(
                    acc[:, b : b + 1], prod[:, 0:64], axis=mybir.AxisListType.X
                )

            red = small.tile([65, 1], F32, tag="red")
            nc.vector.reduce_sum(red[:], acc[:], axis=mybir.AxisListType.X)
            rhs65 = small.tile([65, 1], F32, tag="rhs65")
            nc.vector.tensor_scalar_mul(rhs65[0:64, :], red[0:64, :], 1.0 / NLM)
            nc.vector.tensor_scalar_mul(rhs65[64:65, :], red[64:65, :], 1.0 / NLM)
            nc.vector.tensor_mul(rhs65[64:65, :], rhs65[64:65, :], rhs65[64:65, :])
            nc.vector.tensor_scalar_mul(rhs65[64:65, :], rhs65[64:65, :], -1.0)
            var_ps = sf_p.tile([1, 1], F32, tag="var")
            nc.tensor.matmul(
                var_ps[:], ones_sb[0:65, 0:1], rhs65[:], start=True, stop=True
            )
            sv = small.tile([1, 6], F32, tag="sv")
            nc.vector.tensor_scalar_add(sv[:, 0:1], var_ps[:], EPS)
            _newton_rsqrt(nc, sv[:, 0:1], sv[:, 1:2], sv[:, 2:3], magic_sb[0:1, :])
            nc.vector.tensor_mul(sv[:, 3:4], sv[:, 1:2], gsim_sb[:])
            sb_ps = sf_p.tile([128, 1], F32, tag="sb")
            nc.tensor.matmul(
                sb_ps[:], ones_sb[0:1, 0:128], sv[:, 3:4], start=True, stop=True
            )
            nc.vector.tensor_copy(s_bcast[:], sb_ps[:])

        # ---- phase A: QK -> exp -> PV -> normalize ----
        ue_sb = big_p.tile([C, NB, L], F32, tag="ue")
        stats = small.tile([C, NB * 2 * 6], F32, tag="stats")

        # pv bufs=1 deliberately serializes each batch's PV behind the previous
        # batch's epilogue: the resulting periodic PE idle keeps the power
        # governor oscillating (k=8 windows) instead of pinning the 50% clamp.
        with tc.tile_pool(name="lg", bufs=3, space="PSUM") as lg_p, tc.tile_pool(
            name="pv", bufs=1, space="PSUM"
        ) as pv_p:
            for b in range(NB):
                pair, r = divmod(b, 2)
                pv = pv_p.tile([96, L], F32, tag="pv")
                for c in range(NCH):
                    lg = lg_p.tile([128, L], F32, tag="lg")
                    for j in range(2):
                        nc.tensor.matmul(
                            lg[:, j * 512 : (j + 1) * 512],
                            k2_sb[r * 64 : r * 64 + 64, pair, c * 128 : (c + 1) * 128],
                            q2_sb[r * 64 : r * 64 + 64, pair, j * 512 : (j + 1) * 512],
                            start=True,
                            stop=True,
                        )
                    pt = pt_p.tile([128, L], BF16, tag="pt")
                    nc.scalar.activation(pt[:], lg[:], AF.Exp, scale=s_bcast[:, 0:1])
                    for j in range(2):
                        nc.tensor.matmul(
                            pv[0:66, j * 512 : (j + 1) * 512],
                            vt_sb[:, b, c, :],
                            pt[:, j * 512 : (j + 1) * 512],
                            start=(c == 0),
                            stop=(c == NCH - 1),
                            skip_group_check=True,
                        )
                # epilogue: rows 64-65 of pv hold the softmax denominators
                # (two ones columns in V1^T). Broadcast row 64 to partitions
                # 0-63 with two cross-quadrant stream_shuffles straight from
                # PSUM, reciprocal in place, normalize, BN stats. Pure DVE.
                # The last batch runs in column halves so its serial chain
                # (which gates the whole tail) is half as long.
                bcast = bc_p.tile([64, L], F32, tag="bcast")
                halves = (
                    [(0, L)]
                    if b < NB - 1
                    else [(0, L // 2), (L // 2, L)]
                )
                for hi, (h0, h1) in enumerate(halves):
                    nc.vector.stream_shuffle(
                        bcast[0:32, h0:h1], pv[64:96, h0:h1], MASK0
                    )
                    nc.vector.stream_shuffle(
                        bcast[32:64, h0:h1], pv[64:96, h0:h1], MASK0
                    )
                    nc.vector.reciprocal_approx_fast(
                        out=bcast[0:64, h0:h1], in_=bcast[0:64, h0:h1]
                    )
                    nc.vector.tensor_mul(
                        ue_sb[:, b, h0:h1], pv[0:64, h0:h1], bcast[0:64, h0:h1]
                    )
                    if len(halves) == 2:
                        nc.vector.bn_stats(
                            stats[:, (b * 2 + hi) * 6 : (b * 2 + hi + 1) * 6],
                            ue_sb[:, b, h0:h1],
                        )
                if len(halves) == 1:
                    for half in range(2):
                        nc.vector.bn_stats(
                            stats[:, (b * 2 + half) * 6 : (b * 2 + half + 1) * 6],
                            ue_sb[:, b, half * 512 : (half + 1) * 512],
                        )

        # preload the GELU act table while the val-BN chain runs
        nc.scalar.activation(dume[:, 1:2], dum[:, 1:2], AF.Gelu)

        # ---- phase B: val-BN affine + gelu + store ----
        chan = small.tile([C, 2], F32, tag="chan")
        nc.vector.bn_aggr(chan[:], stats[:])
        vb = small.tile([C, 6], F32, tag="vb")
        nc.vector.tensor_scalar_add(vb[:, 0:1], chan[:, 1:2], EPS)
        _newton_rsqrt(nc, vb[:, 0:1], vb[:, 1:2], vb[:, 2:3], magic_sb[:, :], iters=2)
        a_c = small.tile([C, 1], F32, tag="a_c")
        nc.vector.tensor_mul(a_c[:], gval_sb[:], vb[:, 1:2])
        b_c = small.tile([C, 1], F32, tag="b_c")
        nc.vector.tensor_mul(vb[:, 3:4], chan[:, 0:1], a_c[:])
        nc.vector.tensor_sub(b_c[:], bval_sb[:], vb[:, 3:4])

        out_sb = big_p.tile([C, NB, L], F32, tag="outsb")
        for b in range(NB):
            nc.scalar.activation(
                out_sb[:, b, :],
                ue_sb[:, b, :],
                AF.Gelu,
                scale=a_c[:, 0:1],
                bias=b_c[:, 0:1],
            )
            nc.sync.dma_start(out=out_d[b], in_=out_sb[:, b, :])

    nc.compile()
    return nc


def make_in_map(q, k, v, gamma_sim, beta_sim, gamma_val, beta_val, h):
    """Build the per-core (per-head) input map. Layout-only host prep."""
    bf = ml_dtypes.bfloat16
    qh = q[:, h]
    kh = k[:, h]
    vh = v[:, h]

    def p2(x):
        # [NB, 64, L] -> [128, NB//2, L] (batches 2p/2p+1 stacked on partitions)
        return np.ascontiguousarray(
            x.reshape(NB // 2, 128, L).transpose(1, 0, 2)
        ).astype(bf)

    def t1(x, w=65):
        # [NB, 64, L] -> [128, NB, NCH*w] transposed with ones columns 64..w-1
        xt = x.transpose(0, 2, 1)
        out = np.ones((NB, L, w), dtype=np.float32)
        out[:, :, :64] = xt
        out = out.reshape(NB, NCH, 128, w).transpose(2, 0, 1, 3)
        return np.ascontiguousarray(out.reshape(128, NB, NCH * w)).astype(bf)

    def t8(x):
        # [NB, 64, L] -> [128, NB, NCH*65] fp8 with ones column
        xt = x.transpose(0, 2, 1)
        out = np.ones((NB, L, 65), dtype=np.float32)
        out[:, :, :64] = xt
        out = out.reshape(NB, NCH, 128, 65).transpose(2, 0, 1, 3)
        return np.ascontiguousarray(out.reshape(128, NB, NCH * 65)).astype(
            ml_dtypes.float8_e4m3fn
        )

    return {
        "q2": p2(qh),
        "k2": p2(kh),
        "qt1": t8(qh),
        "kt1": t8(kh),
        "vt1": t1(vh, w=66),
        "g_sim": np.asarray(gamma_sim[h], dtype=np.float32).reshape(1, 1),
        "gamma_val": np.asarray(
            gamma_val[h * C : (h + 1) * C], dtype=np.float32
        ).reshape(C, 1),
        "beta_val": np.asarray(
            beta_val[h * C : (h + 1) * C], dtype=np.float32
        ).reshape(C, 1),
    }


_CACHED_NC = None


def _setup_profiling():
    """Make run_bass_kernel_spmd(trace=True) work on images missing
    antenv.axon_hooks: inject the ctypes NTFF hook + keep artifacts local."""
    import contextlib
    import ctypes
    import types

    try:
        from antenv.axon_hooks import get_axon_ntff_profile_hook  # noqa: F401
    except ImportError:
        so_path = os.environ.get("AXON_PJRT_SO", "/opt/axon/libaxon_pjrt.so")
        lib = ctypes.CDLL(so_path)
        lib.axon_start_nrt_profile.argtypes = [
            ctypes.POINTER(ctypes.c_int64),
            ctypes.c_size_t,
        ]
        lib.axon_start_nrt_profile.restype = ctypes.c_int64
        lib.axon_stop_nrt_profile.argtypes = [ctypes.c_char_p]
        lib.axon_stop_nrt_profile.restype = ctypes.c_int64

        @contextlib.contextmanager
        def _hook(output_dir, device_ids):
            import jax

            jax.devices()
            if device_ids:
                ids = (ctypes.c_int64 * len(device_ids))(*device_ids)
                rc = lib.axon_start_nrt_profile(ids, len(device_ids))
            else:
                rc = lib.axon_start_nrt_profile(None, 0)
            if rc != 0:
                raise RuntimeError(f"axon_start_nrt_profile rc={rc}")
            try:
                yield
            finally:
                n = lib.axon_stop_nrt_profile(str(output_dir).encode())
                print(f"ntff profile: {n} file(s) -> {output_dir}", file=sys.stderr)

        mod = types.ModuleType("antenv.axon_hooks")
        mod.get_axon_ntff_profile_hook = lambda: _hook
        mod.set_axon_ntff_profile_hook = lambda h: None
        import antenv

        sys.modules["antenv.axon_hooks"] = mod
        antenv.axon_hooks = mod

    import concourse.bass_utils as bu

    bu.upload_artifacts = lambda tmpdir: f"local://{tmpdir}"


def kernel(query, key, value, gamma_sim, beta_sim, gamma_val, beta_val):
    global _CACHED_NC
    from concourse.bass_utils import run_bass_kernel_spmd

    query = np.asarray(query, dtype=np.float32)
    key = np.asarray(key, dtype=np.float32)
    value = np.asarray(value, dtype=np.float32)
    gamma_sim = np.asarray(gamma_sim, dtype=np.float32)
    gamma_val = np.asarray(gamma_val, dtype=np.float32)
    beta_val = np.asarray(beta_val, dtype=np.float32)

    if _CACHED_NC is None:
        _CACHED_NC = build_nc()
    nc = _CACHED_NC

    in_maps = [
        make_in_map(query, key, value, gamma_sim, None, gamma_val, beta_val, h)
        for h in range(8)
    ]
    trace = bool(int(os.environ.get("BASS_PROFILE", "0")))
    tmpdir = os.environ.get("BASS_PROFILE_DIR") or None
    if trace:
        try:
            _setup_profiling()
        except Exception as e:  # noqa: BLE001
            print(f"profiling setup failed ({e}); running untraced", file=sys.stderr)
            trace = False
    try:
        res = run_bass_kernel_spmd(
            nc, in_maps, list(range(8)), trace=trace, tmpdir=tmpdir
        )
    except Exception:
        if not trace:
            raise
        print("traced run failed; retrying untraced", file=sys.stderr)
        res = run_bass_kernel_spmd(nc, in_maps, list(range(8)), trace=False)
    if trace and res.exec_time_ns is not None:
        print(f"HW exec time: {res.exec_time_ns} ns")

    out = np.empty((NB, 8 * C, L), dtype=np.float32)
    for h in range(8):
        out[:, h * C : (h + 1) * C, :] = res.results[h]["out"]
    return out


# revision 29
# speedup vs baseline: 1.0059x; 1.0059x over previous
"""Trainium2 Bass kernel for nn_AttentionOperation_32521492365427.

kernel(**inputs) -> np.ndarray, full shapes:
  query/key/value: [8, 8, 64, 1024] f32; gamma_sim/beta_sim: [8];
  gamma_val/beta_val: [512]; output: [8, 512, 1024] f32.

Sharded by HEAD across the 8 NeuronCores (one head per core): both
BatchNorms then have core-local statistics, so there are no collectives.

Per-core math:
 - softmax is shift-invariant => the sim-BN reduces to one per-head scale
   s = gamma_sim / sqrt(var(logits) + EPS); beta/mean drop out.
 - sumsq(logits_b) = sum(Gq_b * Gk_b) over 65x65 Gram matrices
   (ones column appended) => logits variance without a stats pass over
   the 8.4M logits.
 - softmax denominators come free as rows 64-95 of the PV matmul by
   appending 32 ones columns to V^T (the stationary operand); matmul cost
   is column-count-bound so the extra output rows are free.
 - denominator reciprocal is broadcast across partitions with two
   cross-quadrant stream_shuffles straight out of PSUM (pure DVE, f32).
 - val-BN affine + exact (erf) gelu fuse into a single ACT pass,
   split per batch and pipelined with the output DMAs.

All big inputs are pre-cast to bf16 on the host so DMAs are half-size
and issue from the hardware DGE queues (sync + scalar engines).
"""

import os
import sys

sys.path.insert(0, "/opt/trn_rl_repo")

from contextlib import ExitStack

import numpy as np
import ml_dtypes

import concourse.bacc as bacc
import concourse.bass as bass  # noqa: F401
import concourse.tile as tile
from concourse import mybir

F32 = mybir.dt.float32
BF16 = mybir.dt.bfloat16
FP8 = mybir.dt.float8e4
I32 = mybir.dt.int32
AF = mybir.ActivationFunctionType
OP = mybir.AluOpType
DR = mybir.MatmulPerfMode.DoubleRow

EPS = 1e-3
NB = 8
D = 64
C = 64
L = 1024
M = 1024
NCH = M // 128
NLM = float(NB * L * M)
MAGIC = 0x5F3759DF
MASK0 = [0] * 32


def _newton_rsqrt(nc, x, y, t, magic_i32, iters=2):
    """y = 1/sqrt(x) entirely on DVE (bit-trick seed + Newton iters)."""
    xi = x.bitcast(I32)
    yi = y.bitcast(I32)
    nc.vector.tensor_scalar(
        out=yi, in0=xi, scalar1=1, scalar2=None, op0=OP.arith_shift_right
    )
    nc.vector.tensor_tensor(out=yi, in0=magic_i32, in1=yi, op=OP.subtract)
    for _ in range(iters):
        nc.vector.tensor_mul(t, y, y)
        nc.vector.tensor_mul(t, t, x)
        nc.vector.tensor_scalar(
            out=t, in0=t, scalar1=-0.5, scalar2=1.5, op0=OP.mult, op1=OP.add
        )
        nc.vector.tensor_mul(y, y, t)


def build_nc(debug: bool = False):
    nc = bacc.Bacc("TRN2", target_bir_lowering=False, debug=debug)

    q2_d = nc.dram_tensor("q2", [128, NB // 2, L], BF16, kind="ExternalInput")
    k2_d = nc.dram_tensor("k2", [128, NB // 2, L], BF16, kind="ExternalInput")
    qt1_d = nc.dram_tensor("qt1", [128, NB, NCH * 65], FP8, kind="ExternalInput")
    kt1_d = nc.dram_tensor("kt1", [128, NB, NCH * 65], FP8, kind="ExternalInput")
    vt1_d = nc.dram_tensor("vt1", [128, NB, NCH * 66], BF16, kind="ExternalInput")
    gsim_d = nc.dram_tensor("g_sim", [1, 1], F32, kind="ExternalInput")
    gval_d = nc.dram_tensor("gamma_val", [C, 1], F32, kind="ExternalInput")
    bval_d = nc.dram_tensor("beta_val", [C, 1], F32, kind="ExternalInput")
    out_d = nc.dram_tensor("out", [NB, C, L], F32, kind="ExternalOutput")

    with tile.TileContext(nc) as tc, ExitStack() as ctx:
        const_p = ctx.enter_context(tc.tile_pool(name="const", bufs=1))
        pt_p = ctx.enter_context(tc.tile_pool(name="pt", bufs=4))
        bc_p = ctx.enter_context(tc.tile_pool(name="bc", bufs=2))
        big_p = ctx.enter_context(tc.tile_pool(name="big", bufs=1))
        small = ctx.enter_context(tc.tile_pool(name="small", bufs=1))

        # ---- constants / input staging ----
        ones_sb = const_p.tile([128, 128], F32, tag="ones")
        nc.vector.memset(ones_sb[:], 1.0)
        ones_bf = const_p.tile([64, 512], BF16, tag="onesbf")
        nc.gpsimd.memset(ones_bf[:], 1.0)
        magic_sb = const_p.tile([C, 1], I32, tag="magic")
        nc.vector.memset(magic_sb[:], MAGIC)
        dum = small.tile([1, 2], F32, tag="dum")
        nc.gpsimd.memset(dum[:], 0.5)

        qt_sb = const_p.tile([128, NB, NCH, 65], FP8, tag="qt")
        kt_sb = const_p.tile([128, NB, NCH, 65], FP8, tag="kt")
        vt_sb = const_p.tile([128, NB, NCH, 66], BF16, tag="vt")
        q2_sb = const_p.tile([128, NB // 2, L], BF16, tag="q2")
        k2_sb = const_p.tile([128, NB // 2, L], BF16, tag="k2")

        H = NB // 2
        # interleave issue engines: sync and scalar are the two HW DGE queues
        nc.sync.dma_start(out=qt_sb[:, 0:4], in_=qt1_d[:, 0:4])
        nc.scalar.dma_start(out=kt_sb[:, 0:4], in_=kt1_d[:, 0:4])
        nc.sync.dma_start(out=q2_sb[:, 0:1, :], in_=q2_d[:, 0:1])
        nc.scalar.dma_start(out=k2_sb[:, 0:1, :], in_=k2_d[:, 0:1])
        nc.sync.dma_start(out=qt_sb[:, 4:8], in_=qt1_d[:, 4:8])
        nc.scalar.dma_start(out=kt_sb[:, 4:8], in_=kt1_d[:, 4:8])
        nc.sync.dma_start(out=vt_sb[:], in_=vt1_d[:])
        nc.scalar.dma_start(out=q2_sb[:, 1:4, :], in_=q2_d[:, 1:4])
        nc.sync.dma_start(out=k2_sb[:, 1:4, :], in_=k2_d[:, 1:4])

        gsim_sb = const_p.tile([1, 1], F32, tag="gsim")
        nc.scalar.dma_start(out=gsim_sb[:], in_=gsim_d[:])
        gval_sb = const_p.tile([C, 1], F32, tag="gval")
        nc.scalar.dma_start(out=gval_sb[:], in_=gval_d[:])
        bval_sb = const_p.tile([C, 1], F32, tag="bval")
        nc.scalar.dma_start(out=bval_sb[:], in_=bval_d[:])

        # preload the EXP act table while DMAs are in flight
        dume = small.tile([1, 2], F32, tag="dume")
        nc.scalar.activation(dume[:, 0:1], dum[:, 0:1], AF.Exp)

        s_bcast = small.tile([128, 1], F32, tag="sbc")

        # ---- phase 0: Gram-matrix logits variance -> s ----
        with tc.tile_pool(name="gram", bufs=3, space="PSUM") as gram_p, tc.tile_pool(
            name="sf", bufs=1, space="PSUM"
        ) as sf_p:
            acc = small.tile([65, NB], F32, tag="acc")
            for b in range(NB):
                gq_ps = gram_p.tile([65, 65], F32, tag="gq")
                gk_ps = gram_p.tile([65, 65], F32, tag="gk")
                for c in range(NCH):
                    nc.tensor.matmul(
                        gq_ps[:],
                        qt_sb[:, b, c, :],
                        qt_sb[:, b, c, :],
                        start=(c == 0),
                        stop=(c == NCH - 1),
                    )
                for c in range(NCH):
                    nc.tensor.matmul(
                        gk_ps[:],
                        kt_sb[:, b, c, :],
                        kt_sb[:, b, c, :],
                        start=(c == 0),
                        stop=(c == NCH - 1),
                    )
                gq_sb = small.tile([65, 65], F32, tag="gq_sb")
                nc.scalar.copy(gq_sb[:], gq_ps[:])
                gk_sb = small.tile([65, 65], F32, tag="gk_sb")
                nc.vector.tensor_copy(gk_sb[:], gk_ps[:])
                prod = small.tile([65, 65], F32, tag="prod")
                nc.vector.tensor_mul(prod[:], gq_sb[:], gk_sb[:])
                nc.vector.reduce_sum(
                    acc[:, b : b + 1], prod[:, 0:64], axis=mybir.AxisListType.X
                )

            red = small.tile([65, 1], F32, tag="red")
            nc.vector.reduce_sum(red[:], acc[:], axis=mybir.AxisListType.X)
            rhs65 = small.tile([65, 1], F32, tag="rhs65")
            nc.vector.tensor_scalar_mul(rhs65[0:64, :], red[0:64, :], 1.0 / NLM)
            nc.vector.tensor_scalar_mul(rhs65[64:65, :], red[64:65, :], 1.0 / NLM)
            nc.vector.tensor_mul(rhs65[64:65, :], rhs65[64:65, :], rhs65[64:65, :])
            nc.vector.tensor_scalar_mul(rhs65[64:65, :], rhs65[64:65, :], -1.0)
            var_ps = sf_p.tile([1, 1], F32, tag="var")
            nc.tensor.matmul(
                var_ps[:], ones_sb[0:65, 0:1], rhs65[:], start=True, stop=True
            )
            sv = small.tile([1, 6], F32, tag="sv")
            nc.vector.tensor_scalar_add(sv[:, 0:1], var_ps[:], EPS)
            _newton_rsqrt(nc, sv[:, 0:1], sv[:, 1:2], sv[:, 2:3], magic_sb[0:1, :])
            nc.vector.tensor_mul(sv[:, 3:4], sv[:, 1:2], gsim_sb[:])
            sb_ps = sf_p.tile([128, 1], F32, tag="sb")
            nc.tensor.matmul(
                sb_ps[:], ones_sb[0:1, 0:128], sv[:, 3:4], start=True, stop=True
            )
            nc.vector.tensor_copy(s_bcast[:], sb_ps[:])

        # ---- phase A: QK -> exp -> PV -> normalize ----
        ue_sb = big_p.tile([C, NB, L], F32, tag="ue")
        stats = small.tile([C, NB * 2 * 6], F32, tag="stats")

        # pv bufs=1 deliberately serializes each batch's PV behind the previous
        # batch's epilogue: the resulting periodic PE idle keeps the power
        # governor oscillating (k=8 windows) instead of pinning the 50% clamp.
        with tc.tile_pool(name="lg", bufs=3, space="PSUM") as lg_p, tc.tile_pool(
            name="pv", bufs=1, space="PSUM"
        ) as pv_p:
            for b in range(NB):
                pair, r = divmod(b, 2)
                pv = pv_p.tile([96, L], F32, tag="pv")
                for c in range(NCH):
                    lg = lg_p.tile([128, L], F32, tag="lg")
                    for j in range(2):
                        nc.tensor.matmul(
                            lg[:, j * 512 : (j + 1) * 512],
                            k2_sb[r * 64 : r * 64 + 64, pair, c * 128 : (c + 1) * 128],
                            q2_sb[r * 64 : r * 64 + 64, pair, j * 512 : (j + 1) * 512],
                            start=True,
                            stop=True,
                        )
                    pt = pt_p.tile([128, L], BF16, tag="pt")
                    nc.scalar.activation(pt[:], lg[:], AF.Exp, scale=s_bcast[:, 0:1])
                    for j in range(2):
                        nc.tensor.matmul(
                            pv[0:66, j * 512 : (j + 1) * 512],
                            vt_sb[:, b, c, :],
                            pt[:, j * 512 : (j + 1) * 512],
                            start=(c == 0),
                            stop=(c == NCH - 1),
                            skip_group_check=True,
                        )
                # epilogue: rows 64-65 of pv hold the softmax denominators
                # (two ones columns in V1^T). Broadcast row 64 to partitions
                # 0-63 with two cross-quadrant stream_shuffles straight from
                # PSUM, reciprocal in place, normalize, BN stats. Pure DVE.
                # The last batch runs in column halves so its serial chain
                # (which gates the whole tail) is half as long.
                bcast = bc_p.tile([64, L], F32, tag="bcast")
                nc.vector.stream_shuffle(bcast[0:32, :], pv[64:96, :], MASK0)
                nc.vector.stream_shuffle(bcast[32:64, :], pv[64:96, :], MASK0)
                nc.vector.reciprocal_approx_fast(
                    out=bcast[0:64, :], in_=bcast[0:64, :]
                )
                nc.vector.tensor_mul(ue_sb[:, b, :], pv[0:64, :], bcast[0:64, :])
                for half in range(2):
                    nc.vector.bn_stats(
                        stats[:, (b * 2 + half) * 6 : (b * 2 + half + 1) * 6],
                        ue_sb[:, b, half * 512 : (half + 1) * 512],
                    )

        # preload the GELU act table while the val-BN chain runs
        nc.scalar.activation(dume[:, 1:2], dum[:, 1:2], AF.Gelu)

        # ---- phase B: val-BN affine + gelu + store ----
        chan = small.tile([C, 2], F32, tag="chan")
        nc.vector.bn_aggr(chan[:], stats[:])
        vb = small.tile([C, 6], F32, tag="vb")
        nc.vector.tensor_scalar_add(vb[:, 0:1], chan[:, 1:2], EPS)
        _newton_rsqrt(nc, vb[:, 0:1], vb[:, 1:2], vb[:, 2:3], magic_sb[:, :], iters=2)
        a_c = small.tile([C, 1], F32, tag="a_c")
        nc.vector.tensor_mul(a_c[:], gval_sb[:], vb[:, 1:2])
        b_c = small.tile([C, 1], F32, tag="b_c")
        nc.vector.tensor_mul(vb[:, 3:4], chan[:, 0:1], a_c[:])
        nc.vector.tensor_sub(b_c[:], bval_sb[:], vb[:, 3:4])

        out_sb = big_p.tile([C, NB, L], F32, tag="outsb")
        for b in range(NB):
            nc.scalar.activation(
                out_sb[:, b, :],
                ue_sb[:, b, :],
                AF.Gelu,
                scale=a_c[:, 0:1],
                bias=b_c[:, 0:1],
            )
            nc.sync.dma_start(out=out_d[b], in_=out_sb[:, b, :])

    nc.compile()
    return nc


def make_in_map(q, k, v, gamma_sim, beta_sim, gamma_val, beta_val, h):
    """Build the per-core (per-head) input map. Layout-only host prep."""
    bf = ml_dtypes.bfloat16
    qh = q[:, h]
    kh = k[:, h]
    vh = v[:, h]

    def p2(x):
        # [NB, 64, L] -> [128, NB//2, L] (batches 2p/2p+1 stacked on partitions)
        return np.ascontiguousarray(
            x.reshape(NB // 2, 128, L).transpose(1, 0, 2)
        ).astype(bf)

    def t1(x, w=65):
        # [NB, 64, L] -> [128, NB, NCH*w] transposed with ones columns 64..w-1
        xt = x.transpose(0, 2, 1)
        out = np.ones((NB, L, w), dtype=np.float32)
        out[:, :, :64] = xt
        out = out.reshape(NB, NCH, 128, w).transpose(2, 0, 1, 3)
        return np.ascontiguousarray(out.reshape(128, NB, NCH * w)).astype(bf)

    def t8(x):
        # [NB, 64, L] -> [128, NB, NCH*65] fp8 with ones column
        xt = x.transpose(0, 2, 1)
        out = np.ones((NB, L, 65), dtype=np.float32)
        out[:, :, :64] = xt
        out = out.reshape(NB, NCH, 128, 65).transpose(2, 0, 1, 3)
        return np.ascontiguousarray(out.reshape(128, NB, NCH * 65)).astype(
            ml_dtypes.float8_e4m3fn
        )

    return {
        "q2": p2(qh),
        "k2": p2(kh),
        "qt1": t8(qh),
        "kt1": t8(kh),
        "vt1": t1(vh, w=66),
        "g_sim": np.asarray(gamma_sim[h], dtype=np.float32).reshape(1, 1),
        "gamma_val": np.asarray(
            gamma_val[h * C : (h + 1) * C], dtype=np.float32
        ).reshape(C, 1),
        "beta_val": np.asarray(
            beta_val[h * C : (h + 1) * C], dtype=np.float32
        ).reshape(C, 1),
    }


_CACHED_NC = None


def _setup_profiling():
    """Make run_bass_kernel_spmd(trace=True) work on images missing
    antenv.axon_hooks: inject the ctypes NTFF hook + keep artifacts local."""
    import contextlib
    import ctypes
    import types

    try:
        from antenv.axon_hooks import get_axon_ntff_profile_hook  # noqa: F401
    except ImportError:
        so_path = os.environ.get("AXON_PJRT_SO", "/opt/axon/libaxon_pjrt.so")
        lib = ctypes.CDLL(so_path)
        lib.axon_start_nrt_profile.argtypes = [
            ctypes.POINTER(ctypes.c_int64),
            ctypes.c_size_t,
        ]
        lib.axon_start_nrt_profile.restype = ctypes.c_int64
        lib.axon_stop_nrt_profile.argtypes = [ctypes.c_char_p]
        lib.axon_stop_nrt_profile.restype = ctypes.c_int64

        @contextlib.contextmanager
        def _hook(output_dir, device_ids):
            import jax

            jax.devices()
            if device_ids:
                ids = (ctypes.c_int64 * len(device_ids))(*device_ids)
                rc = lib.axon_start_nrt_profile(ids, len(device_ids))
            else:
                rc = lib.axon_start_nrt_profile(None, 0)
            if rc != 0:
                raise RuntimeError(f"axon_start_nrt_profile rc={rc}")
            try:
                yield
            finally:
                n = lib.axon_stop_nrt_profile(str(output_dir).encode())
                print(f"ntff profile: {n} file(s) -> {output_dir}", file=sys.stderr)

        mod = types.ModuleType("antenv.axon_hooks")
        mod.get_axon_ntff_profile_hook = lambda: _hook
        mod.set_axon_ntff_profile_hook = lambda h: None
        import antenv

        sys.modules["antenv.axon_hooks"] = mod
        antenv.axon_hooks = mod

    import concourse.bass_utils as bu

    bu.upload_artifacts = lambda tmpdir: f"local://{tmpdir}"


def kernel(query, key, value, gamma_sim, beta_sim, gamma_val, beta_val):
    global _CACHED_NC
    from concourse.bass_utils import run_bass_kernel_spmd

    query = np.asarray(query, dtype=np.float32)
    key = np.asarray(key, dtype=np.float32)
    value = np.asarray(value, dtype=np.float32)
    gamma_sim = np.asarray(gamma_sim, dtype=np.float32)
    gamma_val = np.asarray(gamma_val, dtype=np.float32)
    beta_val = np.asarray(beta_val, dtype=np.float32)

    if _CACHED_NC is None:
        _CACHED_NC = build_nc()
    nc = _CACHED_NC

    in_maps = [
        make_in_map(query, key, value, gamma_sim, None, gamma_val, beta_val, h)
        for h in range(8)
    ]
    trace = bool(int(os.environ.get("BASS_PROFILE", "0")))
    tmpdir = os.environ.get("BASS_PROFILE_DIR") or None
    if trace:
        try:
            _setup_profiling()
        except Exception as e:  # noqa: BLE001
            print(f"profiling setup failed ({e}); running untraced", file=sys.stderr)
            trace = False
    try:
        res = run_bass_kernel_spmd(
            nc, in_maps, list(range(8)), trace=trace, tmpdir=tmpdir
        )
    except Exception:
        if not trace:
            raise
        print("traced run failed; retrying untraced", file=sys.stderr)
        res = run_bass_kernel_spmd(nc, in_maps, list(range(8)), trace=False)
    if trace and res.exec_time_ns is not None:
        print(f"HW exec time: {res.exec_time_ns} ns")

    out = np.empty((NB, 8 * C, L), dtype=np.float32)
    for h in range(8):
        out[:, h * C : (h + 1) * C, :] = res.results[h]["out"]
    return out


# revision 40
# speedup vs baseline: 1.0738x; 1.0676x over previous
"""Trainium2 Bass kernel for nn_AttentionOperation_32521492365427.

kernel(**inputs) -> np.ndarray, full shapes:
  query/key/value: [8, 8, 64, 1024] f32; gamma_sim/beta_sim: [8];
  gamma_val/beta_val: [512]; output: [8, 512, 1024] f32.

Sharded by HEAD across the 8 NeuronCores (one head per core): both
BatchNorms then have core-local statistics, so there are no collectives.

Per-core math:
 - softmax is shift-invariant => the sim-BN reduces to one per-head scale
   s = gamma_sim / sqrt(var(logits) + EPS); beta/mean drop out.
 - sumsq(logits_b) = sum(Gq_b * Gk_b) over 65x65 Gram matrices
   (ones column appended) => logits variance without a stats pass over
   the 8.4M logits. The Gram pass runs on fp8 copies of Q/K (variance
   statistics are insensitive to 3-mantissa-bit quantization).
 - softmax denominators come free as rows 64-65 of the PV matmul by
   appending two ones columns to V^T (the stationary operand); matmul
   cost is column-count-bound so the extra output rows are free.
 - denominator reciprocal is broadcast across partitions with two
   cross-quadrant stream_shuffles straight out of PSUM (pure DVE, f32).
 - val-BN affine + exact (erf) gelu fuse into a single ACT pass,
   split per batch and pipelined with the output DMAs.

Scheduling is shaped around the TRN2 power governor, which clamps engine
utilization to 50% (HAM k=4/8) whenever sustained activity exceeds its
budget, and only releases during sufficiently idle windows. A fully
saturated PE stream gets pinned at the clamp (positive feedback: the
backlog keeps busy% at 100%), so the per-batch PV is deliberately
single-buffered: each batch's PV waits on the previous epilogue, giving
the PE a ~2us idle slice per batch that keeps the governor oscillating
(~75% effective duty instead of 50%). lg=3/pt=5 buffers were tuned
empirically around that equilibrium.

All big inputs are pre-cast to bf16/fp8 on the host so DMAs are half
size and issue from the hardware DGE queues (sync + scalar engines).
"""

import os
import sys

sys.path.insert(0, "/opt/trn_rl_repo")

from contextlib import ExitStack

import numpy as np
import ml_dtypes

import concourse.bacc as bacc
import concourse.bass as bass  # noqa: F401
import concourse.tile as tile
from concourse import mybir

F32 = mybir.dt.float32
BF16 = mybir.dt.bfloat16
FP8 = mybir.dt.float8e4
I32 = mybir.dt.int32
AF = mybir.ActivationFunctionType
OP = mybir.AluOpType

EPS = 1e-3
NB = 8
D = 64
C = 64
L = 1024
M = 1024
NCH = M // 128
NLM = float(NB * L * M)
MAGIC = 0x5F3759DF
MASK0 = [0] * 32


def _newton_rsqrt(nc, x, y, t, magic_i32, iters=2):
    """y = 1/sqrt(x) entirely on DVE (bit-trick seed + Newton iters)."""
    xi = x.bitcast(I32)
    yi = y.bitcast(I32)
    nc.vector.tensor_scalar(
        out=yi, in0=xi, scalar1=1, scalar2=None, op0=OP.arith_shift_right
    )
    nc.vector.tensor_tensor(out=yi, in0=magic_i32, in1=yi, op=OP.subtract)
    for _ in range(iters):
        nc.vector.tensor_mul(t, y, y)
        nc.vector.tensor_mul(t, t, x)
        nc.vector.tensor_scalar(
            out=t, in0=t, scalar1=-0.5, scalar2=1.5, op0=OP.mult, op1=OP.add
        )
        nc.vector.tensor_mul(y, y, t)


def build_nc(debug: bool = False):
    nc = bacc.Bacc("TRN2", target_bir_lowering=False, debug=debug)

    q2_d = nc.dram_tensor("q2", [128, NB // 2, L], BF16, kind="ExternalInput")
    k2_d = nc.dram_tensor("k2", [128, NB // 2, L], BF16, kind="ExternalInput")
    qt1_d = nc.dram_tensor("qt1", [128, NB, NCH * 65], FP8, kind="ExternalInput")
    kt1_d = nc.dram_tensor("kt1", [128, NB, NCH * 65], FP8, kind="ExternalInput")
    vt1_d = nc.dram_tensor("vt1", [128, NB, NCH * 66], BF16, kind="ExternalInput")
    gsim_d = nc.dram_tensor("g_sim", [1, 1], F32, kind="ExternalInput")
    gval_d = nc.dram_tensor("gamma_val", [C, 1], F32, kind="ExternalInput")
    bval_d = nc.dram_tensor("beta_val", [C, 1], F32, kind="ExternalInput")
    out_d = nc.dram_tensor("out", [NB, C, L], F32, kind="ExternalOutput")

    with tile.TileContext(nc) as tc, ExitStack() as ctx:
        const_p = ctx.enter_context(tc.tile_pool(name="const", bufs=1))
        pt_p = ctx.enter_context(tc.tile_pool(name="pt", bufs=5))
        bc_p = ctx.enter_context(tc.tile_pool(name="bc", bufs=1))
        big_p = ctx.enter_context(tc.tile_pool(name="big", bufs=1))
        small = ctx.enter_context(tc.tile_pool(name="small", bufs=2))

        # ---- constants / input staging ----
        ones_sb = const_p.tile([128, 128], F32, tag="ones")
        nc.vector.memset(ones_sb[:], 1.0)
        ones_bf = const_p.tile([64, 512], BF16, tag="onesbf")
        nc.gpsimd.memset(ones_bf[:], 1.0)
        magic_sb = const_p.tile([C, 1], I32, tag="magic")
        nc.vector.memset(magic_sb[:], MAGIC)
        dum = small.tile([1, 2], F32, tag="dum")
        nc.gpsimd.memset(dum[:], 0.5)

        qt_sb = const_p.tile([128, NB, NCH, 65], FP8, tag="qt")
        kt_sb = const_p.tile([128, NB, NCH, 65], FP8, tag="kt")
        vt_sb = const_p.tile([128, NB, NCH, 66], BF16, tag="vt")
        q2_sb = const_p.tile([128, NB // 2, L], BF16, tag="q2")
        k2_sb = const_p.tile([128, NB // 2, L], BF16, tag="k2")

        # interleave issue engines: sync and scalar are the two HW DGE queues.
        # qt/kt gate the Gram pass (and thus s and the first exp) - issue both
        # halves of each before anything else.
        nc.sync.dma_start(out=qt_sb[:, 0:4], in_=qt1_d[:, 0:4])
        nc.scalar.dma_start(out=kt_sb[:, 0:4], in_=kt1_d[:, 0:4])
        nc.sync.dma_start(out=qt_sb[:, 4:8], in_=qt1_d[:, 4:8])
        nc.scalar.dma_start(out=kt_sb[:, 4:8], in_=kt1_d[:, 4:8])
        nc.sync.dma_start(out=q2_sb[:, 0:1, :], in_=q2_d[:, 0:1])
        nc.scalar.dma_start(out=k2_sb[:, 0:1, :], in_=k2_d[:, 0:1])
        nc.sync.dma_start(out=vt_sb[:], in_=vt1_d[:])
        nc.scalar.dma_start(out=q2_sb[:, 1:4, :], in_=q2_d[:, 1:4])
        nc.sync.dma_start(out=k2_sb[:, 1:4, :], in_=k2_d[:, 1:4])

        gsim_sb = const_p.tile([1, 1], F32, tag="gsim")
        nc.scalar.dma_start(out=gsim_sb[:], in_=gsim_d[:])
        gval_sb = const_p.tile([C, 1], F32, tag="gval")
        nc.scalar.dma_start(out=gval_sb[:], in_=gval_d[:])
        bval_sb = const_p.tile([C, 1], F32, tag="bval")
        nc.scalar.dma_start(out=bval_sb[:], in_=bval_d[:])

        # preload the EXP act table while DMAs are in flight
        dume = small.tile([1, 2], F32, tag="dume")
        nc.scalar.activation(dume[:, 0:1], dum[:, 0:1], AF.Exp)

        s_bcast = small.tile([128, 1], F32, tag="sbc")

        # ---- phase 0: Gram-matrix logits variance -> s ----
        with tc.tile_pool(name="gram", bufs=3, space="PSUM") as gram_p, tc.tile_pool(
            name="sf", bufs=1, space="PSUM"
        ) as sf_p:
            acc = small.tile([65, NB], F32, tag="acc")
            for b in range(NB):
                gq_ps = gram_p.tile([65, 65], F32, tag="gq")
                gk_ps = gram_p.tile([65, 65], F32, tag="gk")
                for c in range(NCH):
                    nc.tensor.matmul(
                        gq_ps[:],
                        qt_sb[:, b, c, :],
                        qt_sb[:, b, c, :],
                        start=(c == 0),
                        stop=(c == NCH - 1),
                    )
                for c in range(NCH):
                    nc.tensor.matmul(
                        gk_ps[:],
                        kt_sb[:, b, c, :],
                        kt_sb[:, b, c, :],
                        start=(c == 0),
                        stop=(c == NCH - 1),
                    )
                gq_sb = small.tile([65, 65], F32, tag="gq_sb")
                nc.scalar.copy(gq_sb[:], gq_ps[:])
                gk_sb = small.tile([65, 65], F32, tag="gk_sb")
                nc.vector.tensor_copy(gk_sb[:], gk_ps[:])
                prod = small.tile([65, 65], F32, tag="prod")
                nc.vector.tensor_mul(prod[:], gq_sb[:], gk_sb[:])
                nc.vector.reduce_sum(
                    acc[:, b : b + 1], prod[:, 0:64], axis=mybir.AxisListType.X
                )

            red = small.tile([65, 1], F32, tag="red")
            nc.vector.reduce_sum(red[:], acc[:], axis=mybir.AxisListType.X)
            rhs65 = small.tile([65, 1], F32, tag="rhs65")
            nc.vector.tensor_scalar_mul(rhs65[0:64, :], red[0:64, :], 1.0 / NLM)
            nc.vector.tensor_scalar_mul(rhs65[64:65, :], red[64:65, :], 1.0 / NLM)
            nc.vector.tensor_mul(rhs65[64:65, :], rhs65[64:65, :], rhs65[64:65, :])
            nc.vector.tensor_scalar_mul(rhs65[64:65, :], rhs65[64:65, :], -1.0)
            var_ps = sf_p.tile([1, 1], F32, tag="var")
            nc.tensor.matmul(
                var_ps[:], ones_sb[0:65, 0:1], rhs65[:], start=True, stop=True
            )
            sv = small.tile([1, 6], F32, tag="sv")
            nc.vector.tensor_scalar_add(sv[:, 0:1], var_ps[:], EPS)
            _newton_rsqrt(nc, sv[:, 0:1], sv[:, 1:2], sv[:, 2:3], magic_sb[0:1, :])
            nc.vector.tensor_mul(sv[:, 3:4], sv[:, 1:2], gsim_sb[:])
            sb_ps = sf_p.tile([128, 1], F32, tag="sb")
            nc.tensor.matmul(
                sb_ps[:], ones_sb[0:1, 0:128], sv[:, 3:4], start=True, stop=True
            )
            nc.vector.tensor_copy(s_bcast[:], sb_ps[:])

        # ---- phase A: QK -> exp -> PV -> normalize ----
        ue_sb = big_p.tile([C, NB, L], F32, tag="ue")
        stats = small.tile([C, NB * 2 * 6], F32, tag="stats")

        # pv bufs=1 deliberately serializes each batch's PV behind the previous
        # batch's epilogue: the resulting periodic PE idle keeps the power
        # governor oscillating (k=8 windows) instead of pinning the 50% clamp.
        with tc.tile_pool(name="lg", bufs=3, space="PSUM") as lg_p, tc.tile_pool(
            name="pv", bufs=1, space="PSUM"
        ) as pv_p:

            def emit_qk_exp(b, c):
                pair, r = divmod(b, 2)
                lg = lg_p.tile([128, L], F32, tag="lg")
                for j in range(2):
                    nc.tensor.matmul(
                        lg[:, j * 512 : (j + 1) * 512],
                        k2_sb[r * 64 : r * 64 + 64, pair, c * 128 : (c + 1) * 128],
                        q2_sb[r * 64 : r * 64 + 64, pair, j * 512 : (j + 1) * 512],
                        start=True,
                        stop=True,
                    )
                pt = pt_p.tile([128, L], BF16, tag="pt")
                nc.scalar.activation(pt[:], lg[:], AF.Exp, scale=s_bcast[:, 0:1])
                return pt

            pending_pt = None
            for b in range(NB):
                pv = pv_p.tile([96, L], F32, tag="pv")
                for c in range(NCH):
                    if c == 0 and pending_pt is not None:
                        pt = pending_pt
                        pending_pt = None
                    else:
                        pt = emit_qk_exp(b, c)
                    if c == NCH - 1 and b + 1 < NB:
                        # pre-emit the next batch's first QK+exp ahead of this
                        # batch's final PV so the scalar engine's exp stream
                        # has no batch-boundary bubble
                        pending_pt = emit_qk_exp(b + 1, 0)
                    for j in range(2):
                        nc.tensor.matmul(
                            pv[0:66, j * 512 : (j + 1) * 512],
                            vt_sb[:, b, c, :],
                            pt[:, j * 512 : (j + 1) * 512],
                            start=(c == 0),
                            stop=(c == NCH - 1),
                            skip_group_check=True,
                        )
                # epilogue: rows 64-65 of pv hold the softmax denominators
                # (two ones columns in V1^T). Broadcast row 64 to partitions
                # 0-63 with two cross-quadrant stream_shuffles straight from
                # PSUM, reciprocal in place, normalize, BN stats. Pure DVE.
                # The last batch runs in column halves so its serial chain
                # (which gates the whole tail) is half as long.
                bcast = bc_p.tile([64, L], F32, tag="bcast")
                nc.vector.stream_shuffle(bcast[0:32, :], pv[64:96, :], MASK0)
                nc.vector.stream_shuffle(bcast[32:64, :], pv[64:96, :], MASK0)
                nc.vector.reciprocal_approx_fast(
                    out=bcast[0:64, :], in_=bcast[0:64, :]
                )
                nc.vector.tensor_mul(ue_sb[:, b, :], pv[0:64, :], bcast[0:64, :])
                for half in range(2):
                    nc.vector.bn_stats(
                        stats[:, (b * 2 + half) * 6 : (b * 2 + half + 1) * 6],
                        ue_sb[:, b, half * 512 : (half + 1) * 512],
                    )

        # preload the GELU act table while the val-BN chain runs
        nc.scalar.activation(dume[:, 1:2], dum[:, 1:2], AF.Gelu)

        # ---- phase B: val-BN affine + gelu + store ----
        chan = small.tile([C, 2], F32, tag="chan")
        nc.vector.bn_aggr(chan[:], stats[:])
        vb = small.tile([C, 6], F32, tag="vb")
        nc.vector.tensor_scalar_add(vb[:, 0:1], chan[:, 1:2], EPS)
        _newton_rsqrt(nc, vb[:, 0:1], vb[:, 1:2], vb[:, 2:3], magic_sb[:, :], iters=1)
        a_c = small.tile([C, 1], F32, tag="a_c")
        nc.vector.tensor_mul(a_c[:], gval_sb[:], vb[:, 1:2])
        b_c = small.tile([C, 1], F32, tag="b_c")
        nc.vector.tensor_mul(vb[:, 3:4], chan[:, 0:1], a_c[:])
        nc.vector.tensor_sub(b_c[:], bval_sb[:], vb[:, 3:4])

        # alternate output DMAs across both HW DGE queues (sync + scalar) so
        # the 2MB of stores drain in parallel instead of serializing on one
        # ~220GB/s queue. Scalar issues its DMAs between its gelu passes.
        out_sb = big_p.tile([C, NB, L], F32, tag="outsb")
        for p in range(NB // 2):
            b = 2 * p
            nc.scalar.activation(
                out_sb[:, b : b + 2, :],
                ue_sb[:, b : b + 2, :],
                AF.Gelu,
                scale=a_c[:, 0:1],
                bias=b_c[:, 0:1],
            )
            eng = nc.sync if p % 2 == 0 else nc.scalar
            eng.dma_start(
                out=out_d[b : b + 2].transpose([1, 0, 2]),
                in_=out_sb[:, b : b + 2, :],
            )

    nc.compile()
    return nc


def make_in_map(q, k, v, gamma_sim, beta_sim, gamma_val, beta_val, h):
    """Build the per-core (per-head) input map. Layout-only host prep."""
    bf = ml_dtypes.bfloat16
    qh = q[:, h]
    kh = k[:, h]
    vh = v[:, h]

    def p2(x):
        # [NB, 64, L] -> [128, NB//2, L] (batches 2p/2p+1 stacked on partitions)
        return np.ascontiguousarray(
            x.reshape(NB // 2, 128, L).transpose(1, 0, 2)
        ).astype(bf)

    def t1(x, w=65):
        # [NB, 64, L] -> [128, NB, NCH*w] transposed with ones columns 64..w-1
        xt = x.transpose(0, 2, 1)
        out = np.ones((NB, L, w), dtype=np.float32)
        out[:, :, :64] = xt
        out = out.reshape(NB, NCH, 128, w).transpose(2, 0, 1, 3)
        return np.ascontiguousarray(out.reshape(128, NB, NCH * w)).astype(bf)

    def t8(x):
        # [NB, 64, L] -> [128, NB, NCH*65] fp8 with ones column
        xt = x.transpose(0, 2, 1)
        out = np.ones((NB, L, 65), dtype=np.float32)
        out[:, :, :64] = xt
        out = out.reshape(NB, NCH, 128, 65).transpose(2, 0, 1, 3)
        return np.ascontiguousarray(out.reshape(128, NB, NCH * 65)).astype(
            ml_dtypes.float8_e4m3fn
        )

    return {
        "q2": p2(qh),
        "k2": p2(kh),
        "qt1": t8(qh),
        "kt1": t8(kh),
        "vt1": t1(vh, w=66),
        "g_sim": np.asarray(gamma_sim[h], dtype=np.float32).reshape(1, 1),
        "gamma_val": np.asarray(
            gamma_val[h * C : (h + 1) * C], dtype=np.float32
        ).reshape(C, 1),
        "beta_val": np.asarray(
            beta_val[h * C : (h + 1) * C], dtype=np.float32
        ).reshape(C, 1),
    }


_CACHED_NC = None


def _setup_profiling():
    """Make run_bass_kernel_spmd(trace=True) work on images missing
    antenv.axon_hooks: inject the ctypes NTFF hook + keep artifacts local."""
    import contextlib
    import ctypes
    import types

    try:
        from antenv.axon_hooks import get_axon_ntff_profile_hook  # noqa: F401
    except ImportError:
        so_path = os.environ.get("AXON_PJRT_SO", "/opt/axon/libaxon_pjrt.so")
        lib = ctypes.CDLL(so_path)
        lib.axon_start_nrt_profile.argtypes = [
            ctypes.POINTER(ctypes.c_int64),
            ctypes.c_size_t,
        ]
        lib.axon_start_nrt_profile.restype = ctypes.c_int64
        lib.axon_stop_nrt_profile.argtypes = [ctypes.c_char_p]
        lib.axon_stop_nrt_profile.restype = ctypes.c_int64

        @contextlib.contextmanager
        def _hook(output_dir, device_ids):
            import jax

            jax.devices()
            if device_ids:
                ids = (ctypes.c_int64 * len(device_ids))(*device_ids)
                rc = lib.axon_start_nrt_profile(ids, len(device_ids))
            else:
                rc = lib.axon_start_nrt_profile(None, 0)
            if rc != 0:
                raise RuntimeError(f"axon_start_nrt_profile rc={rc}")
            try:
                yield
            finally:
                n = lib.axon_stop_nrt_profile(str(output_dir).encode())
                print(f"ntff profile: {n} file(s) -> {output_dir}", file=sys.stderr)

        mod = types.ModuleType("antenv.axon_hooks")
        mod.get_axon_ntff_profile_hook = lambda: _hook
        mod.set_axon_ntff_profile_hook = lambda h: None
        import antenv

        sys.modules["antenv.axon_hooks"] = mod
        antenv.axon_hooks = mod

    import concourse.bass_utils as bu

    bu.upload_artifacts = lambda tmpdir: f"local://{tmpdir}"


def kernel(query, key, value, gamma_sim, beta_sim, gamma_val, beta_val):
    global _CACHED_NC
    from concourse.bass_utils import run_bass_kernel_spmd

    query = np.asarray(query, dtype=np.float32)
    key = np.asarray(key, dtype=np.float32)
    value = np.asarray(value, dtype=np.float32)
    gamma_sim = np.asarray(gamma_sim, dtype=np.float32)
    gamma_val = np.asarray(gamma_val, dtype=np.float32)
    beta_val = np.asarray(beta_val, dtype=np.float32)

    if _CACHED_NC is None:
        _CACHED_NC = build_nc()
    nc = _CACHED_NC

    in_maps = [
        make_in_map(query, key, value, gamma_sim, None, gamma_val, beta_val, h)
        for h in range(8)
    ]
    trace = bool(int(os.environ.get("BASS_PROFILE", "0")))
    tmpdir = os.environ.get("BASS_PROFILE_DIR") or None
    if trace:
        try:
            _setup_profiling()
        except Exception as e:  # noqa: BLE001
            print(f"profiling setup failed ({e}); running untraced", file=sys.stderr)
            trace = False
    try:
        res = run_bass_kernel_spmd(
            nc, in_maps, list(range(8)), trace=trace, tmpdir=tmpdir
        )
    except Exception:
        if not trace:
            raise
        print("traced run failed; retrying untraced", file=sys.stderr)
        res = run_bass_kernel_spmd(nc, in_maps, list(range(8)), trace=False)
    if trace and res.exec_time_ns is not None:
        print(f"HW exec time: {res.exec_time_ns} ns")

    out = np.empty((NB, 8 * C, L), dtype=np.float32)
    for h in range(8):
        out[:, h * C : (h + 1) * C, :] = res.results[h]["out"]
    return out
